# revision 8
# baseline (speedup 1.0000x reference)
"""Trainium2 Bass kernel for nn_NonLocalDenoiser (LIDIA Aggregation0, top-1 self
neighbor): weighted patch fold -> normalize on device; unfold replicated on
host (pure indexing, same class as the baseline's host transposes).

Key hardware fact (measured): HBM<->SBUF DMA runs at ~285 GB/s only when the
SBUF AP spans all 128 partitions ([64,*] ~215 GB/s; ragged counts like 82 fall
to ~45 GB/s). So both input and output are shipped as [128, *] tiles.

A frame (156 patch rows) is split into 3 slabs of 64 input rows
(a in {0, 48, 92}); a pair-task stacks two slabs in the partition dim
[128, COLS]. The dy-fold matmul uses one lhsT per dy that simultaneously maps
slab A rows p -> packed row q = p + dy (canvas rows 0..63) and slab B rows
p -> q = p + dy - 4 (canvas rows 4..67 at q 64..127). This requires A to
never need canvas rows 64..67 (A in {top, mid}) and B to never need rows
0..3 (B in {mid, bot}); tops->A (28), bots->B (28), mids split 12/12/4
(4 mids ride the per-core single-slab task).

Device pipeline per task:
  - DMA [128, COLS]: 75 feature blocks + 1 dist block, each
    [4-col zero pad | 156 data], +4 tail cols
  - ACT: w = exp(-d) in place
  - DVE: features *= w (broadcast); s[(dy,c)] = sum_dx w*x[(c,dy,dx)]
    col-shifted; bw = box_x(w) replicated to the 5 dy slots
  - PE: 10 matmuls: psC[128,480] (img) and psW[128,160] (wimg) packed fold
  - DVE: rimg = 1/wimg; ob = img*rimg
  - DMA out [128, 480]
Host: assemble nimg[28,160,160,3], as_strided unfold, final transpose.
"""
import numpy as np

PS, C, NH, W = 5, 3, 156, 160
T, HORF, VF = 28, 14, 75
SLAB = 64            # input patch rows per slab
CV = SLAB + 4        # canvas rows per slab
NB = VF + 1          # 75 feature blocks + 1 w block
COLS = NB * W + 4    # 12164
NPAIR = 5            # pair tasks per core
NCORES = 8
NPATCH = NH * NH
CW = C * W
# band -> (a, first used canvas row, last+1); img row = a + canvas row
BANDS = ((0, 0, 64), (48, 16, 64), (92, 20, 68))

LAST_EXEC_NS = None


def _assignment():
    """A-slabs (40), B-slabs (40), singles (8, None = dummy)."""
    tops = [(tau, 0) for tau in range(T)]
    mids = [(tau, 1) for tau in range(T)]
    bots = [(tau, 2) for tau in range(T)]
    a_list = tops + mids[12:24]
    b_list = bots + mids[0:12]
    singles = mids[24:28] + [None] * 4
    return a_list, b_list, singles


def _build_program(loop_reps=1, do_out=True, do_mm=True, do_dve=True,
                   do_in=True):
    import contextlib
    import concourse.bass as bass
    import concourse.bacc as bacc
    import concourse.mybir as mybir
    import concourse.tile as tile

    f32 = mybir.dt.float32
    nc = bacc.Bacc(None)
    XP = nc.declare_dram_parameter("xp", [NPAIR, 128, COLS], f32, isOutput=False)
    X1 = nc.declare_dram_parameter("x1", [SLAB, COLS], f32, isOutput=False)
    OP = nc.declare_dram_parameter("op", [NPAIR, 128, CW], f32, isOutput=True)
    O1 = nc.declare_dram_parameter("o1", [SLAB, CW], f32, isOutput=True)
    WB = VF * W          # w block column base
    SW = 4 * W           # S tile: per-dy group (c0,c1,c2,bw) * 160

    with tile.TileContext(nc) as tc:
        with tc.tile_pool(name="const", bufs=1) as cpool, \
             tc.tile_pool(name="xsp", bufs=2) as xpool, \
             tc.tile_pool(name="ssp", bufs=2) as spool, \
             tc.tile_pool(name="osp", bufs=2) as opool, \
             tc.tile_pool(name="ps", bufs=2, space="PSUM") as ppool:
            # M_dy [128,128]: cols 0..63 (A): q == p + dy; cols 64..127 (B):
            # q == p + dy - 4 (slice-local j: j == p + dy - 68)
            ids = cpool.tile([128, 5 * 128], f32)
            nc.gpsimd.memset(ids[:], 0.0)
            for dy in range(PS):
                sl = ids[:, dy * 128:dy * 128 + SLAB]
                nc.gpsimd.affine_select(
                    out=sl, in_=sl, pattern=[[-1, SLAB]],
                    compare_op=mybir.AluOpType.not_equal, fill=1.0,
                    base=dy, channel_multiplier=1)
                sl = ids[:, dy * 128 + SLAB:(dy + 1) * 128]
                nc.gpsimd.affine_select(
                    out=sl, in_=sl, pattern=[[-1, SLAB]],
                    compare_op=mybir.AluOpType.not_equal, fill=1.0,
                    base=dy - CV, channel_multiplier=1)

            loop_cm = (tc.For_i(0, loop_reps) if loop_reps > 1
                       else contextlib.nullcontext())
            with loop_cm:
              for j in range(NPAIR + 1):
                single = j == NPAIR
                xt = xpool.tile([128, COLS], f32, tag="x")
                xfull = xt[:]
                xpitch = xfull.ap[0][0]
                if do_in:
                    if single:
                        nc.sync.dma_start(out=xt[0:SLAB, :], in_=X1[:])
                    else:
                        nc.sync.dma_start(out=xfull, in_=XP[j])
                # w = exp(-d) on the dist block's data cols
                wdat = xt[:, WB + 4:WB + W]
                nc.scalar.activation(out=wdat, in_=wdat,
                                     func=mybir.ActivationFunctionType.Exp,
                                     scale=-1.0)
                st = spool.tile([128, PS * SW], f32, tag="s")
                sfull = st[:]
                spitch = sfull.ap[0][0]
                if do_dve:
                    # features *= w, one 3D op per channel (25 planes each)
                    wb = xt[:, WB:WB + W]
                    wb25 = wb.unsqueeze(1).to_broadcast([128, 25, W])
                    for c in range(C):
                        f3 = xt[:, c * 25 * W:(c + 1) * 25 * W].rearrange(
                            "p (v q) -> p v q", q=W)
                        nc.gpsimd.tensor_tensor(out=f3, in0=f3, in1=wb25,
                                                op=mybir.AluOpType.mult)
                    # s[(dy,c)] = sum_dx wx[(c,dy,dx)] col-shifted by 4-dx
                    s_c = bass.AP(sfull.tensor, sfull.offset,
                                  [[spitch, 128], [SW, PS], [W, C], [1, W]])

                    def wx_ap(k):
                        return bass.AP(xfull.tensor, xfull.offset + 159 * k + 4,
                                       [[xpitch, 128], [PS * W, PS],
                                        [25 * W, C], [1, W]])

                    nc.vector.tensor_tensor(out=s_c, in0=wx_ap(0), in1=wx_ap(1),
                                            op=mybir.AluOpType.add)
                    for k in (2, 3, 4):
                        nc.vector.tensor_tensor(out=s_c, in0=s_c, in1=wx_ap(k),
                                                op=mybir.AluOpType.add)
                    # bw = box_x(w) into dy=0 slot, then replicate to dy=1..4
                    bw0 = st[:, CW:SW]
                    nc.gpsimd.tensor_tensor(out=bw0, in0=xt[:, WB + 4:WB + 4 + W],
                                            in1=xt[:, WB + 3:WB + 3 + W],
                                            op=mybir.AluOpType.add)
                    for off in (2, 1, 0):
                        nc.gpsimd.tensor_tensor(
                            out=bw0, in0=bw0, in1=xt[:, WB + off:WB + off + W],
                            op=mybir.AluOpType.add)
                    rep_out = bass.AP(sfull.tensor, sfull.offset + SW + CW,
                                      [[spitch, 128], [SW, 4], [1, W]])
                    nc.scalar.copy(out=rep_out,
                                   in_=bw0.unsqueeze(1).to_broadcast([128, 4, W]))

                ob = opool.tile([128, CW], f32, tag="ob")
                rt = opool.tile([128, W], f32, tag="r")
                psC = ppool.tile([128, CW], f32, tag="psC", space="PSUM")
                psW = ppool.tile([128, W], f32, tag="psW", space="PSUM")
                if do_mm:
                    for dy in range(PS):
                        lhsT = ids[:, dy * 128:(dy + 1) * 128]
                        nc.tensor.matmul(out=psC[:], lhsT=lhsT,
                                         rhs=st[:, dy * SW:dy * SW + CW],
                                         start=(dy == 0), stop=(dy == 4))
                        nc.tensor.matmul(out=psW[:], lhsT=lhsT,
                                         rhs=st[:, dy * SW + CW:(dy + 1) * SW],
                                         start=(dy == 0), stop=(dy == 4))
                    nc.vector.reciprocal(out=rt[:], in_=psW[:])
                    nc.vector.tensor_tensor(
                        out=ob[:].rearrange("p (c q) -> p c q", q=W),
                        in0=psC[:].rearrange("p (c q) -> p c q", q=W),
                        in1=rt[:].unsqueeze(1).to_broadcast([128, C, W]),
                        op=mybir.AluOpType.mult)
                if do_out and do_mm:
                    if single:
                        nc.gpsimd.dma_start(out=O1[:], in_=ob[0:SLAB, :])
                    else:
                        nc.gpsimd.dma_start(out=OP[j], in_=ob[:])
    nc.finalize()
    return nc


def _host_prep(x, nlDists):
    # xt[tau, hi, v, wi] = x[i, hi*156+wi, f, v], tau = i*14+f
    xt = np.ascontiguousarray(
        x.reshape(2, NH, NH, HORF, VF).transpose(0, 3, 1, 4, 2)
    ).reshape(T, NH, VF, NH)
    d = np.ascontiguousarray(nlDists[:, :, 0]).reshape(T, NH, NH)
    # per-patch-row canvas rows: 76 blocks of [4 zero | 156 data] + 4 tail
    ROWS = np.zeros((T, NH, COLS), np.float32)
    rv = ROWS[:, :, :NB * W].reshape(T, NH, NB, W)
    rv[:, :, :VF, 4:] = xt
    rv[:, :, VF, 4:] = d
    a_list, b_list, singles = _assignment()
    XPa = np.zeros((NCORES, NPAIR, 128, COLS), np.float32)
    X1a = np.zeros((NCORES, SLAB, COLS), np.float32)
    for c in range(NCORES):
        for jj in range(NPAIR):
            ta, ba = a_list[c * NPAIR + jj]
            tb, bb = b_list[c * NPAIR + jj]
            XPa[c, jj, 0:SLAB] = ROWS[ta, BANDS[ba][0]:BANDS[ba][0] + SLAB]
            XPa[c, jj, SLAB:128] = ROWS[tb, BANDS[bb][0]:BANDS[bb][0] + SLAB]
        if singles[c] is not None:
            ts, bs = singles[c]
            X1a[c] = ROWS[ts, BANDS[bs][0]:BANDS[bs][0] + SLAB]
    return [{"xp": XPa[c], "x1": X1a[c]} for c in range(NCORES)]


def _host_post(OPa, O1a):
    # OPa [8,5,128,480], O1a [8,64,480] -> (2, 24336, 14, 75)
    a_list, b_list, singles = _assignment()
    nimg = np.empty((T, W, W, C), np.float32)

    def put(tau, band, rows, base):
        # rows: packed [64, CW]; packed row q holds canvas row q + base
        a, r0, r1 = BANDS[band]
        rr = rows.reshape(SLAB, C, W)
        nimg[tau, a + r0:a + r1] = rr[r0 - base:r1 - base].transpose(0, 2, 1)

    for c in range(NCORES):
        for jj in range(NPAIR):
            ta, ba = a_list[c * NPAIR + jj]
            tb, bb = b_list[c * NPAIR + jj]
            put(ta, ba, OPa[c, jj, 0:SLAB], 0)
            put(tb, bb, OPa[c, jj, SLAB:128], 4)
        if singles[c] is not None:
            ts, bs = singles[c]
            put(ts, bs, O1a[c], 0)

    st = nimg.strides
    out6 = np.lib.stride_tricks.as_strided(
        nimg, (T, NH, NH, C, PS, PS),
        (st[0], st[1], st[2], st[3], st[1], st[2]))
    out_flat = out6.reshape(T, NPATCH, VF)
    return np.ascontiguousarray(
        out_flat.reshape(2, HORF, VF, NPATCH).transpose(0, 3, 1, 2))


def _is_self_inds(nlInds):
    k0 = np.asarray(nlInds)[:, :, 0, :]
    j = np.arange(NPATCH)
    return (bool((k0[:, :, 0] == np.arange(T, dtype=k0.dtype)[:, None]).all())
            and bool((k0[:, :, 1] == (j // NH).astype(k0.dtype)).all())
            and bool((k0[:, :, 2] == (j % NH).astype(k0.dtype)).all()))


def _numpy_fallback(x, nlDists, nlInds, H, Wp):
    images, patches, hor_f, ver_f = x.shape
    t = images * hor_f
    N = t * patches
    xr = np.transpose(x, (0, 2, 3, 1)).reshape(t, ver_f, patches)
    pat = np.transpose(xr, (0, 2, 1)).reshape(N, C, PS, PS)
    w = np.exp(-nlDists[:, :, 0].reshape(N))
    inds = nlInds[:, :, 0, :].reshape(N, 3)
    ti, hi, wi = inds[:, 0], inds[:, 1], inds[:, 2]
    dd = np.arange(PS)
    sidx = (ti[:, None, None] * (H * Wp)
            + (hi[:, None, None] + dd[None, :, None]) * Wp
            + (wi[:, None, None] + dd[None, None, :])).reshape(-1)
    vals = (w[:, None, None, None] * pat).transpose(0, 2, 3, 1).reshape(-1, C)
    img = np.zeros((t * H * Wp, C), x.dtype)
    np.add.at(img, sidx, vals)
    wimg = np.zeros((t * H * Wp,), x.dtype)
    np.add.at(wimg, sidx, np.repeat(w, PS * PS))
    img = img / wimg[:, None]
    out_pat = img[sidx].reshape(N, PS, PS, C).transpose(0, 3, 1, 2)
    out = out_pat.reshape(t, patches, ver_f)
    return np.ascontiguousarray(
        out.reshape(images, hor_f, ver_f, patches).transpose(0, 3, 1, 2))


def kernel(x, nlDists, nlInds, pixels_h, pixels_w):
    global LAST_EXEC_NS
    import os
    x = np.asarray(x, np.float32)
    nlDists = np.asarray(nlDists, np.float32)
    if (x.shape != (2, NPATCH, HORF, VF) or int(pixels_h) != 160
            or int(pixels_w) != 160 or not _is_self_inds(nlInds)):
        return _numpy_fallback(np.asarray(x), np.asarray(nlDists),
                               np.asarray(nlInds), int(pixels_h), int(pixels_w))

    from concourse.bass_utils import run_bass_kernel_spmd
    in_maps = _host_prep(x, nlDists)
    nc = _build_program()
    trace = bool(os.environ.get("BASS_KERNEL_PROFILE"))
    res = run_bass_kernel_spmd(nc, in_maps, list(range(NCORES)), trace=trace)
    LAST_EXEC_NS = res.exec_time_ns
    OPa = np.stack([np.asarray(res.results[c]["op"], np.float32)
                    for c in range(NCORES)])
    O1a = np.stack([np.asarray(res.results[c]["o1"], np.float32)
                    for c in range(NCORES)])
    return _host_post(OPa, O1a)


# revision 9
# speedup vs baseline: 1.4771x; 1.4771x over previous
"""Trainium2 Bass kernel for nn_NonLocalDenoiser (LIDIA Aggregation0, top-1 self
neighbor): weighted patch fold -> normalize on device; unfold replicated on
host (pure indexing, same class as the baseline's host transposes).

Key hardware fact (measured): HBM<->SBUF DMA runs at ~285 GB/s only when the
SBUF AP spans all 128 partitions ([64,*] ~215 GB/s; ragged counts like 82 fall
to ~45 GB/s). So both input and output are shipped as [128, *] tiles.

A frame (156 patch rows) is split into 3 slabs of 64 input rows
(a in {0, 48, 92}); a pair-task stacks two slabs in the partition dim
[128, COLS]. The dy-fold matmul uses one lhsT per dy that simultaneously maps
slab A rows p -> packed row q = p + dy (canvas rows 0..63) and slab B rows
p -> q = p + dy - 4 (canvas rows 4..67 at q 64..127). This requires A to
never need canvas rows 64..67 (A in {top, mid}) and B to never need rows
0..3 (B in {mid, bot}); tops->A (28), bots->B (28), mids split 12/12/4
(4 mids ride the per-core single-slab task).

Device pipeline per task:
  - DMA [128, COLS]: 75 feature blocks + 1 dist block, each
    [4-col zero pad | 156 data], +4 tail cols
  - ACT: w = exp(-d) in place
  - DVE: features *= w (broadcast); s[(dy,c)] = sum_dx w*x[(c,dy,dx)]
    col-shifted; bw = box_x(w) replicated to the 5 dy slots
  - PE: 10 matmuls: psC[128,480] (img) and psW[128,160] (wimg) packed fold
  - DVE: rimg = 1/wimg; ob = img*rimg
  - DMA out [128, 480]
Host: assemble nimg[28,160,160,3], as_strided unfold, final transpose.
"""
import numpy as np

PS, C, NH, W = 5, 3, 156, 160
T, HORF, VF = 28, 14, 75
SLAB = 64            # input patch rows per slab
CV = SLAB + 4        # canvas rows per slab
NB = VF + 1          # 75 feature blocks + 1 w block
COLS = NB * W + 4    # 12164
NPAIR = 5            # pair tasks per core
NCORES = 8
NPATCH = NH * NH
CW = C * W
# band -> (a, first used canvas row, last+1); img row = a + canvas row
BANDS = ((0, 0, 64), (48, 16, 64), (92, 20, 68))

LAST_EXEC_NS = None


def _assignment():
    """A-slabs (40), B-slabs (40), singles (8, None = dummy)."""
    tops = [(tau, 0) for tau in range(T)]
    mids = [(tau, 1) for tau in range(T)]
    bots = [(tau, 2) for tau in range(T)]
    a_list = tops + mids[12:24]
    b_list = bots + mids[0:12]
    singles = mids[24:28] + [None] * 4
    return a_list, b_list, singles


def _build_program(loop_reps=1, do_out=True, do_mm=True, do_dve=True,
                   do_in=True):
    import contextlib
    import concourse.bass as bass
    import concourse.bacc as bacc
    import concourse.mybir as mybir
    import concourse.tile as tile

    f32 = mybir.dt.float32
    nc = bacc.Bacc(None)
    XP = nc.declare_dram_parameter("xp", [NPAIR, 128, COLS], f32, isOutput=False)
    X1 = nc.declare_dram_parameter("x1", [SLAB, COLS], f32, isOutput=False)
    OP = nc.declare_dram_parameter("op", [NPAIR, 128, CW], f32, isOutput=True)
    O1 = nc.declare_dram_parameter("o1", [SLAB, CW], f32, isOutput=True)
    WB = VF * W          # w block column base
    SW = 4 * W           # S tile: per-dy group (c0,c1,c2,bw) * 160

    with tile.TileContext(nc) as tc:
        with tc.tile_pool(name="const", bufs=1) as cpool, \
             tc.tile_pool(name="xsp", bufs=2) as xpool, \
             tc.tile_pool(name="ssp", bufs=2) as spool, \
             tc.tile_pool(name="osp", bufs=2) as opool, \
             tc.tile_pool(name="ps", bufs=2, space="PSUM") as ppool:
            # M_dy [128,128]: cols 0..63 (A): q == p + dy; cols 64..127 (B):
            # q == p + dy - 4 (slice-local j: j == p + dy - 68)
            ids = cpool.tile([128, 5 * 128], f32)
            nc.gpsimd.memset(ids[:], 0.0)
            for dy in range(PS):
                sl = ids[:, dy * 128:dy * 128 + SLAB]
                nc.gpsimd.affine_select(
                    out=sl, in_=sl, pattern=[[-1, SLAB]],
                    compare_op=mybir.AluOpType.not_equal, fill=1.0,
                    base=dy, channel_multiplier=1)
                sl = ids[:, dy * 128 + SLAB:(dy + 1) * 128]
                nc.gpsimd.affine_select(
                    out=sl, in_=sl, pattern=[[-1, SLAB]],
                    compare_op=mybir.AluOpType.not_equal, fill=1.0,
                    base=dy - CV, channel_multiplier=1)

            loop_cm = (tc.For_i(0, loop_reps) if loop_reps > 1
                       else contextlib.nullcontext())
            with loop_cm:
              for j in range(NPAIR + 1):
                single = j == NPAIR
                xt = xpool.tile([128, COLS], f32, tag="x")
                xfull = xt[:]
                xpitch = xfull.ap[0][0]
                if do_in:
                    if single:
                        nc.sync.dma_start(out=xt[0:SLAB, :], in_=X1[:])
                    else:
                        nc.sync.dma_start(out=xfull, in_=XP[j])
                # w25: 25 copies of exp(-d), block layout [4 pad | 156]
                w25 = spool.tile([128, 25 * W], f32, tag="w25")
                w25v = w25[:]
                wpitch = w25v.ap[0][0]
                pad_ap = bass.AP(w25v.tensor, w25v.offset,
                                 [[wpitch, 128], [W, 25], [1, 4]])
                nc.scalar.memzero(pad_ap)
                dat25 = bass.AP(w25v.tensor, w25v.offset + 4,
                                [[wpitch, 128], [W, 25], [1, NH]])
                nc.scalar.activation(
                    out=dat25,
                    in_=xt[:, WB + 4:WB + W].unsqueeze(1).to_broadcast(
                        [128, 25, NH]),
                    func=mybir.ActivationFunctionType.Exp, scale=-1.0)
                st = spool.tile([128, PS * SW], f32, tag="s")
                sfull = st[:]
                spitch = sfull.ap[0][0]
                if do_dve:
                    # features *= w, one 2D op per channel (25 planes each)
                    for c in range(C):
                        f2 = xt[:, c * 25 * W:(c + 1) * 25 * W]
                        nc.vector.tensor_tensor(out=f2, in0=f2, in1=w25[:],
                                                op=mybir.AluOpType.mult)
                    # s[(dy,c)] = sum_dx wx[(c,dy,dx)] col-shifted by 4-dx
                    s_c = bass.AP(sfull.tensor, sfull.offset,
                                  [[spitch, 128], [SW, PS], [W, C], [1, W]])

                    def wx_ap(k):
                        return bass.AP(xfull.tensor, xfull.offset + 159 * k + 4,
                                       [[xpitch, 128], [PS * W, PS],
                                        [25 * W, C], [1, W]])

                    nc.vector.tensor_tensor(out=s_c, in0=wx_ap(0), in1=wx_ap(1),
                                            op=mybir.AluOpType.add)
                    for k in (2, 3, 4):
                        nc.vector.tensor_tensor(out=s_c, in0=s_c, in1=wx_ap(k),
                                                op=mybir.AluOpType.add)
                    # bw = box_x(w) into dy=0 slot, then replicate to dy=1..4
                    bw0 = st[:, CW:SW]
                    nc.vector.tensor_tensor(out=bw0, in0=w25[:, 4:4 + W],
                                            in1=w25[:, 3:3 + W],
                                            op=mybir.AluOpType.add)
                    for off in (2, 1, 0):
                        nc.vector.tensor_tensor(
                            out=bw0, in0=bw0, in1=w25[:, off:off + W],
                            op=mybir.AluOpType.add)
                    rep_out = bass.AP(sfull.tensor, sfull.offset + SW + CW,
                                      [[spitch, 128], [SW, 4], [1, W]])
                    nc.scalar.copy(out=rep_out,
                                   in_=bw0.unsqueeze(1).to_broadcast([128, 4, W]))

                ob = opool.tile([128, CW], f32, tag="ob")
                rt = opool.tile([128, W], f32, tag="r")
                psC = ppool.tile([128, CW], f32, tag="psC", space="PSUM")
                psW = ppool.tile([128, W], f32, tag="psW", space="PSUM")
                if do_mm:
                    for dy in range(PS):
                        lhsT = ids[:, dy * 128:(dy + 1) * 128]
                        nc.tensor.matmul(out=psC[:], lhsT=lhsT,
                                         rhs=st[:, dy * SW:dy * SW + CW],
                                         start=(dy == 0), stop=(dy == 4))
                        nc.tensor.matmul(out=psW[:], lhsT=lhsT,
                                         rhs=st[:, dy * SW + CW:(dy + 1) * SW],
                                         start=(dy == 0), stop=(dy == 4))
                    nc.vector.reciprocal(out=rt[:], in_=psW[:])
                    nc.vector.tensor_tensor(
                        out=ob[:].rearrange("p (c q) -> p c q", q=W),
                        in0=psC[:].rearrange("p (c q) -> p c q", q=W),
                        in1=rt[:].unsqueeze(1).to_broadcast([128, C, W]),
                        op=mybir.AluOpType.mult)
                if do_out and do_mm:
                    if single:
                        nc.gpsimd.dma_start(out=O1[:], in_=ob[0:SLAB, :])
                    else:
                        nc.gpsimd.dma_start(out=OP[j], in_=ob[:])
    nc.finalize()
    return nc


def _host_prep(x, nlDists):
    # xt[tau, hi, v, wi] = x[i, hi*156+wi, f, v], tau = i*14+f
    xt = np.ascontiguousarray(
        x.reshape(2, NH, NH, HORF, VF).transpose(0, 3, 1, 4, 2)
    ).reshape(T, NH, VF, NH)
    d = np.ascontiguousarray(nlDists[:, :, 0]).reshape(T, NH, NH)
    # per-patch-row canvas rows: 76 blocks of [4 zero | 156 data] + 4 tail
    ROWS = np.zeros((T, NH, COLS), np.float32)
    rv = ROWS[:, :, :NB * W].reshape(T, NH, NB, W)
    rv[:, :, :VF, 4:] = xt
    rv[:, :, VF, 4:] = d
    a_list, b_list, singles = _assignment()
    XPa = np.zeros((NCORES, NPAIR, 128, COLS), np.float32)
    X1a = np.zeros((NCORES, SLAB, COLS), np.float32)
    for c in range(NCORES):
        for jj in range(NPAIR):
            ta, ba = a_list[c * NPAIR + jj]
            tb, bb = b_list[c * NPAIR + jj]
            XPa[c, jj, 0:SLAB] = ROWS[ta, BANDS[ba][0]:BANDS[ba][0] + SLAB]
            XPa[c, jj, SLAB:128] = ROWS[tb, BANDS[bb][0]:BANDS[bb][0] + SLAB]
        if singles[c] is not None:
            ts, bs = singles[c]
            X1a[c] = ROWS[ts, BANDS[bs][0]:BANDS[bs][0] + SLAB]
    return [{"xp": XPa[c], "x1": X1a[c]} for c in range(NCORES)]


def _host_post(OPa, O1a):
    # OPa [8,5,128,480], O1a [8,64,480] -> (2, 24336, 14, 75)
    a_list, b_list, singles = _assignment()
    nimg = np.empty((T, W, W, C), np.float32)

    def put(tau, band, rows, base):
        # rows: packed [64, CW]; packed row q holds canvas row q + base
        a, r0, r1 = BANDS[band]
        rr = rows.reshape(SLAB, C, W)
        nimg[tau, a + r0:a + r1] = rr[r0 - base:r1 - base].transpose(0, 2, 1)

    for c in range(NCORES):
        for jj in range(NPAIR):
            ta, ba = a_list[c * NPAIR + jj]
            tb, bb = b_list[c * NPAIR + jj]
            put(ta, ba, OPa[c, jj, 0:SLAB], 0)
            put(tb, bb, OPa[c, jj, SLAB:128], 4)
        if singles[c] is not None:
            ts, bs = singles[c]
            put(ts, bs, O1a[c], 0)

    st = nimg.strides
    out6 = np.lib.stride_tricks.as_strided(
        nimg, (T, NH, NH, C, PS, PS),
        (st[0], st[1], st[2], st[3], st[1], st[2]))
    out_flat = out6.reshape(T, NPATCH, VF)
    return np.ascontiguousarray(
        out_flat.reshape(2, HORF, VF, NPATCH).transpose(0, 3, 1, 2))


def _is_self_inds(nlInds):
    k0 = np.asarray(nlInds)[:, :, 0, :]
    j = np.arange(NPATCH)
    return (bool((k0[:, :, 0] == np.arange(T, dtype=k0.dtype)[:, None]).all())
            and bool((k0[:, :, 1] == (j // NH).astype(k0.dtype)).all())
            and bool((k0[:, :, 2] == (j % NH).astype(k0.dtype)).all()))


def _numpy_fallback(x, nlDists, nlInds, H, Wp):
    images, patches, hor_f, ver_f = x.shape
    t = images * hor_f
    N = t * patches
    xr = np.transpose(x, (0, 2, 3, 1)).reshape(t, ver_f, patches)
    pat = np.transpose(xr, (0, 2, 1)).reshape(N, C, PS, PS)
    w = np.exp(-nlDists[:, :, 0].reshape(N))
    inds = nlInds[:, :, 0, :].reshape(N, 3)
    ti, hi, wi = inds[:, 0], inds[:, 1], inds[:, 2]
    dd = np.arange(PS)
    sidx = (ti[:, None, None] * (H * Wp)
            + (hi[:, None, None] + dd[None, :, None]) * Wp
            + (wi[:, None, None] + dd[None, None, :])).reshape(-1)
    vals = (w[:, None, None, None] * pat).transpose(0, 2, 3, 1).reshape(-1, C)
    img = np.zeros((t * H * Wp, C), x.dtype)
    np.add.at(img, sidx, vals)
    wimg = np.zeros((t * H * Wp,), x.dtype)
    np.add.at(wimg, sidx, np.repeat(w, PS * PS))
    img = img / wimg[:, None]
    out_pat = img[sidx].reshape(N, PS, PS, C).transpose(0, 3, 1, 2)
    out = out_pat.reshape(t, patches, ver_f)
    return np.ascontiguousarray(
        out.reshape(images, hor_f, ver_f, patches).transpose(0, 3, 1, 2))


def kernel(x, nlDists, nlInds, pixels_h, pixels_w):
    global LAST_EXEC_NS
    import os
    x = np.asarray(x, np.float32)
    nlDists = np.asarray(nlDists, np.float32)
    if (x.shape != (2, NPATCH, HORF, VF) or int(pixels_h) != 160
            or int(pixels_w) != 160 or not _is_self_inds(nlInds)):
        return _numpy_fallback(np.asarray(x), np.asarray(nlDists),
                               np.asarray(nlInds), int(pixels_h), int(pixels_w))

    from concourse.bass_utils import run_bass_kernel_spmd
    in_maps = _host_prep(x, nlDists)
    nc = _build_program()
    trace = bool(os.environ.get("BASS_KERNEL_PROFILE"))
    res = run_bass_kernel_spmd(nc, in_maps, list(range(NCORES)), trace=trace)
    LAST_EXEC_NS = res.exec_time_ns
    OPa = np.stack([np.asarray(res.results[c]["op"], np.float32)
                    for c in range(NCORES)])
    O1a = np.stack([np.asarray(res.results[c]["o1"], np.float32)
                    for c in range(NCORES)])
    return _host_post(OPa, O1a)
